# revision 11
# baseline (speedup 1.0000x reference)
"""Trainium2 Bass kernel for nn_LiquidS4Layer (S4 DPLR forward).

y = causal_conv(u, K) + D*u, K the length-L SSM kernel from small DPLR params.

Device algorithm (per core, 512 of 4096 batch rows):
  1. Discretize via bilinear transform + Woodbury (A = Lambda - P P^H is
     diagonal + rank-1) -> block-real forms blkA1, blkA0H; Abar pair by mm.
  2. Repeated squaring Abar^(2^k), k<=8; V-chain (Abar^j Bbar) and C-chain
     ((Abar^T)^j Ctilde) by doubling -> scan maps Min/E/G0/W1/Wout/Dq2 and
     the near-field Toeplitz T0 (K row via DRAM shift trick, D on diagonal).
     Aliasing correction is skipped: |eig(Abar)|^L <= 2.3e-2 and the measured
     end-to-end impact is < 2e-4 relative.
  3. Chunked scan (chunk Q=128, stride-2 state passing), computing y^T:
     all matmuls put the small [128,128] map as the stationary operand and
     u/h [128,512] as the moving operand; matmuls are grouped by stationary
     operand (4 chunks per group) and repeated weight loads are elided.
Host side does layout only: packs small params, pre-transposes + bf16-casts
the u shard into a [q, chunk, b] DRAM layout (contiguous per partition),
and relayouts the bf16 y^T shard back to f32 [b, t].

Sharding: u/(y) row-sharded over 8 cores (batch*channel parallel); params
replicated; no collectives.
"""
import os
import numpy as np
import ml_dtypes
from contextlib import ExitStack

import concourse.bass as bass
import concourse.tile as tile
from concourse import mybir
from concourse.bass_utils import run_bass_kernel_spmd

F32 = mybir.dt.float32
BF16 = mybir.dt.bfloat16
ALU = mybir.AluOpType

NCORES = 8
BH, L = 4096, 4096
BC = BH // NCORES       # 512 rows per core
N = 64                  # SSM state size
N2 = 2 * N              # real block state size = 128
Q = 128                 # chunk length
NCH = L // Q            # 32 chunks
NSQ = 8                 # Abar^(2^8) = Abar^256 = Dq^2
NG = NCH // 4           # 8 four-chunk groups

# packed-param layout (f32 row)
PK_LRI, PK_PRI, PK_PIR, PK_NIR = 0, 128, 256, 384
PK_BRI, PK_CRI, PK_E1 = 512, 640, 768
PK_D, PK_LSTEP, PK_ONE = 896, 897, 898
PKLEN = 900

LAST_EXEC_NS = None
LAST_RESULTS = None


def build_program():
    nc = bass.Bass()
    dp = nc.declare_dram_parameter
    uT = dp("uT", [128, NCH * BC], BF16, isOutput=False)
    yT = dp("yT", [128, NCH * BC], BF16, isOutput=True)
    pk = dp("pk", [1, PKLEN], F32, isOutput=False)
    cst = dp("cst", [128, 384], F32, isOutput=False)
    zs = dp("zs", [1, 256], BF16, isOutput=False)

    with TileKernel(nc) as tk:
        tk.build(uT, yT, pk, cst, zs)
    if not int(os.environ.get("KERNEL_NO_LDW_DEDUPE", "0")):
        _dedupe_ldweights(nc)
    _split_multi_waits(nc)
    return nc


def _dedupe_ldweights(nc):
    """Mark matmuls whose stationary operand matches the immediately
    preceding matmul's so codegen skips the redundant LDWEIGHTS."""
    for f in nc.m.functions:
        for blk in f.blocks:
            last = None
            for inst in blk.instructions:
                if not isinstance(inst, mybir.InstMatmult):
                    continue
                w = inst.ins[1]
                if inst.is_transpose or w.dtype in (mybir.dt.float32,
                                                    mybir.dt.float32r):
                    last = None
                    continue
                key = (w.memref, w.offset, str(w.ap))
                if last is not None and key == last:
                    inst.ldweights = False
                last = key



def _split_multi_waits(nc):
    """This toolchain's walrus encodes at most one sync wait per (non-Drain)
    instruction.  Tile can emit several; hoist the extras onto standalone
    EventSemaphore wait instructions inserted just before, on the same
    engine (engines execute their stream in order, so this is equivalent)."""
    ctr = 0
    for f in nc.m.functions:
        for blk in f.blocks:
            out = []
            changed = False
            for inst in blk.instructions:
                si = inst.sync_info
                if si is None:
                    out.append(inst)
                    continue
                waits = list(si.on_wait)
                if len(waits) > 1:
                    cands = [u for u in si.on_update] + [
                        w for w in waits if "DMA" not in w.ant_name]
                    for w in waits[:-1]:
                        ev = mybir.InstEventSemaphore(
                            name=f"I-wsplit-{ctr}", ins=[], outs=[])
                        ctr += 1
                        ev.engine = inst.engine
                        c = cands[0] if cands else w
                        up = mybir.SyncUpdate(
                            sync_type="semaphore", id=c.id, ant_name=c.ant_name,
                            update_mode="sem-add-imm", update_value=0,
                            update_reg=None)
                        ev.sync_info = mybir.SyncInfo(on_wait=[w], on_update=[up])
                        out.append(ev)
                    inst.sync_info = mybir.SyncInfo(
                        on_wait=[waits[-1]], on_update=list(si.on_update))
                    changed = True
                out.append(inst)
            if changed:
                blk.instructions = out


class TileKernel:
    def __init__(self, nc):
        self.nc = nc
        self.ctx = ExitStack()
        self.tc = tile.TileContext(nc)

    def __enter__(self):
        self.ctx.__enter__()
        self.tc.__enter__()
        return self

    def __exit__(self, *a):
        self.ctx.__exit__(*a)
        return self.tc.__exit__(*a)

    def pool(self, name, bufs=1, space="SBUF"):
        return self.ctx.enter_context(
            self.tc.tile_pool(name=name, bufs=bufs, space=space))

    def build(self, uT_d, yT_d, pk_d, cst_d, zs_d):
        nc = self.nc
        v = nc.vector
        s = nc.scalar
        g = nc.gpsimd
        con = self.pool("con", 1)
        pp = self.pool("pp", 1)
        pps_ctx = self.tc.tile_pool(name="pps", bufs=2, space="PSUM")
        pps = pps_ctx.__enter__()

        def T(shape, dt=F32, tag=None):
            return pp.tile(shape, dt, tag=tag, name=tag)

        def C(shape, dt=F32, tag=None):
            return con.tile(shape, dt, tag=tag, name=tag)

        def PS(shape, tag="pp_ps"):
            return pps.tile(shape, F32, tag=tag, name=tag)

        # ---- t=0: zero scratch, ACT table preload, PE warm-up -----------
        dz = C([1, 512], tag="dz")
        v.memset(dz[:], 0.0)
        dact = T([1, 1], tag="dact")
        s.activation(dact[:], dz[0:1, 0:1], mybir.ActivationFunctionType.Exp)

        def dummy_mm(row_ap):
            ps = PS([128, 256], tag="pp_ps")
            nc.tensor.matmul(ps[:], row_ap, dz[0:1, 0:256],
                             start=True, stop=True)

        for _ in range(12):
            dummy_mm(dz[0:1, 0:128])

        # ---- loads ------------------------------------------------------
        pk = C([1, PKLEN], tag="pk")
        nc.sync.dma_start(out=pk[:], in_=pk_d[:])
        cst = C([128, 384], tag="cst")
        nc.sync.dma_start(out=cst[:], in_=cst_d[:])
        ident = cst[:, 0:128]
        ilmu = cst[:, 128:256]
        rev = cst[:, 256:384]
        usb = C([128, NCH, BC], BF16, tag="usb")
        uT_r = uT_d.rearrange("q (i b) -> q i b", b=BC)
        for p in range(4):
            nc.sync.dma_start(out=usb[:, p * 8:(p + 1) * 8, :],
                              in_=uT_r[:, p * 8:(p + 1) * 8, :])
        rev_bf = C([128, 128], BF16, tag="rev_bf")
        s.copy(rev_bf[:], rev[:])

        lri = pk[0:1, PK_LRI:PK_LRI + 128]
        pri = pk[0:1, PK_PRI:PK_PRI + 128]
        pir = pk[0:1, PK_PIR:PK_PIR + 128]
        nir = pk[0:1, PK_NIR:PK_NIR + 128]
        bri = pk[0:1, PK_BRI:PK_BRI + 128]
        cri = pk[0:1, PK_CRI:PK_CRI + 128]
        e1 = pk[0:1, PK_E1:PK_E1 + 128]          # [1]*64 + [0]*64
        dval = pk[0:1, PK_D:PK_D + 1]
        lstep = pk[0:1, PK_LSTEP:PK_LSTEP + 1]
        one11 = pk[0:1, PK_ONE:PK_ONE + 1]

        # ---- param chain: vector spine + scalar leaves ------------------
        delta = T([1, 1], tag="delta")
        s.activation(delta[:], lstep[:], mybir.ActivationFunctionType.Exp)
        hh = T([1, 1], tag="hh")
        v.tensor_scalar_mul(hh[:], delta[:], 0.5)
        hl = T([1, 128], tag="hl")
        v.tensor_scalar_mul(hl[:], lri[:], hh[:])
        dummy_mm(hl[:])
        den = T([1, 128], tag="den")             # 1 - hh*lam
        v.scalar_tensor_tensor(den[:], hl[:], -1.0, e1[:], ALU.mult, ALU.add)
        g1 = T([1, 128], tag="g1")               # 1 + hh*lam
        v.tensor_tensor(g1[:], hl[:], e1[:], ALU.add)
        sq = T([1, 128], tag="sq")
        v.tensor_mul(sq[:], den[:], den[:])
        r2 = T([1, 64], tag="r2")
        v.tensor_add(r2[:], sq[0:1, 0:64], sq[0:1, 64:128])
        rr = T([1, 128], tag="rr")
        v.reciprocal(rr[0:1, 0:64], r2[:])
        v.tensor_copy(rr[0:1, 64:128], rr[0:1, 0:64])
        dummy_mm(den[:])
        d0 = T([1, 128], tag="d0")               # 1/den (conj trick)
        v.tensor_mul(d0[0:1, 0:64], den[0:1, 0:64], rr[0:1, 0:64])
        v.scalar_tensor_tensor(d0[0:1, 64:128], den[0:1, 64:128], -1.0,
                               rr[0:1, 64:128], ALU.mult, ALU.mult)
        psq = T([1, 128], tag="psq")
        v.tensor_mul(psq[:], pri[:], pri[:])
        p2 = T([1, 64], tag="p2")
        v.tensor_add(p2[:], psq[0:1, 0:64], psq[0:1, 64:128])
        m1 = T([1, 128], tag="m1")
        v.tensor_mul(m1[:], d0[:], pri[:])
        m2 = T([1, 128], tag="m2")
        v.tensor_mul(m2[:], d0[:], pir[:])
        dummy_mm(d0[:])
        tri = T([1, 128], tag="tri")             # [tre | tim], t = d0*P
        v.tensor_sub(tri[0:1, 0:64], m1[0:1, 0:64], m1[0:1, 64:128])
        v.tensor_add(tri[0:1, 64:128], m2[0:1, 0:64], m2[0:1, 64:128])
        # a0 rows: a = conj(v), v = conj(P)*d0
        a0r0 = T([1, 128], tag="a0r0")           # [vre | -vim]
        a0r1 = T([1, 128], tag="a0r1")           # [vim | vre]
        v.tensor_add(a0r0[0:1, 0:64], m1[0:1, 0:64], m1[0:1, 64:128])
        v.tensor_sub(a0r1[0:1, 0:64], m2[0:1, 64:128], m2[0:1, 0:64])
        s.mul(a0r0[0:1, 64:128], a0r1[0:1, 0:64], -1.0)
        s.copy(a0r1[0:1, 64:128], a0r0[0:1, 0:64])
        # s_c = 1 + hh * sum(|P|^2 d0)
        pd = T([1, 128], tag="pd")
        sri = T([1, 2], tag="sri")
        v.tensor_mul(pd[0:1, 0:64], d0[0:1, 0:64], p2[:])
        v.tensor_mul(pd[0:1, 64:128], d0[0:1, 64:128], p2[:])
        v.reduce_sum(sri[0:1, 0:1], pd[0:1, 0:64], axis=mybir.AxisListType.X)
        v.reduce_sum(sri[0:1, 1:2], pd[0:1, 64:128], axis=mybir.AxisListType.X)
        dummy_mm(m1[:])
        sc = T([1, 2], tag="sc")                 # [s_re | s_im]
        v.tensor_scalar(sc[0:1, 0:1], sri[0:1, 0:1], hh[:], 1.0,
                        op0=ALU.mult, op1=ALU.add)
        v.tensor_scalar_mul(sc[0:1, 1:2], sri[0:1, 1:2], hh[:])
        sq2 = T([1, 2], tag="sq2")
        v.tensor_mul(sq2[:], sc[:], sc[:])
        s2 = T([1, 1], tag="s2")
        v.tensor_add(s2[:], sq2[0:1, 0:1], sq2[0:1, 1:2])
        s2i = T([1, 1], tag="s2i")
        v.reciprocal(s2i[:], s2[:])
        hsr = T([1, 1], tag="hsr")               # hh*s_re*s2i
        v.scalar_tensor_tensor(hsr[:], sc[0:1, 0:1], hh[:], s2i[:],
                               ALU.mult, ALU.mult)
        hsi = T([1, 1], tag="hsi")               # +hh*s_im*s2i (sign folded below)
        v.scalar_tensor_tensor(hsi[:], sc[0:1, 1:2], hh[:], s2i[:],
                               ALU.mult, ALU.mult)
        # w = hs * t -> b0 rows (b~ = w); hs = hsr - i*hsi
        x2a = T([1, 64], tag="x2a")
        v.tensor_scalar_mul(x2a[:], tri[0:1, 64:128], hsi[:])
        x2b = T([1, 64], tag="x2b")
        v.tensor_scalar_mul(x2b[:], tri[0:1, 0:64], hsi[:])
        dummy_mm(m2[:])
        b0r0 = T([1, 128], tag="b0r0")           # [wre | wim]
        b0r1 = T([1, 128], tag="b0r1")           # [-wim | wre]
        v.scalar_tensor_tensor(b0r0[0:1, 0:64], tri[0:1, 0:64], hsr[:],
                               x2a[:], ALU.mult, ALU.add)
        v.scalar_tensor_tensor(b0r0[0:1, 64:128], tri[0:1, 64:128], hsr[:],
                               x2b[:], ALU.mult, ALU.subtract)
        s.mul(b0r1[0:1, 0:64], b0r0[0:1, 64:128], -1.0)
        s.copy(b0r1[0:1, 64:128], b0r0[0:1, 0:64])
        # a1 rows: a = hh*P ; b1 rows are pri / nir (host-packed)
        a1r0 = T([1, 128], tag="a1r0")
        v.tensor_scalar_mul(a1r0[:], pri[:], hh[:])
        a1r1 = T([1, 128], tag="a1r1")
        s.mul(a1r1[0:1, 0:64], a1r0[0:1, 64:128], -1.0)
        s.copy(a1r1[0:1, 64:128], a1r0[0:1, 0:64])
        # stacked diag rows (scalar leaves)
        grow1 = T([1, 128], tag="grow1")
        s.copy(grow1[0:1, 0:64], g1[0:1, 0:64])
        s.copy(grow1[0:1, 64:128], g1[0:1, 0:64])
        girow1 = T([1, 128], tag="girow1")
        s.copy(girow1[0:1, 0:64], g1[0:1, 64:128])
        s.copy(girow1[0:1, 64:128], g1[0:1, 64:128])
        grow0 = T([1, 128], tag="grow0")
        s.copy(grow0[0:1, 0:64], d0[0:1, 0:64])
        s.copy(grow0[0:1, 64:128], d0[0:1, 0:64])
        girow0 = T([1, 128], tag="girow0")
        s.mul(girow0[0:1, 0:64], d0[0:1, 64:128], -1.0)
        s.copy(girow0[0:1, 64:128], girow0[0:1, 0:64])
        brow_s = T([1, 128], tag="brow_s")
        v.tensor_scalar_mul(brow_s[:], bri[:], delta[:])

        # ---- column extraction (one PSUM bank) --------------------------
        psc = PS([128, 8])
        rows = [grow1, girow1, grow0, girow0, brow_s]
        for j, rt in enumerate(rows):
            nc.tensor.matmul(psc[:, j:j + 1], rt[:], one11[:],
                             start=True, stop=True)
        nc.tensor.matmul(psc[:, 5:6], cri[:], one11[:], start=True, stop=True)
        cols = T([128, 6], tag="cols")
        v.tensor_copy(cols[:], psc[:, 0:6])

        # ---- block matrices --------------------------------------------
        def blk_build(tag, gcol, gicol, ar0, ar1, br0, br1):
            dg = T([128, 128], tag=tag + "_dg")
            v.tensor_scalar_mul(dg[:], ident[:], gcol)
            dgi = T([128, 128], tag=tag + "_dgi")
            v.tensor_scalar_mul(dgi[:], ilmu[:], gicol)
            v.tensor_add(dg[:], dg[:], dgi[:])
            ps = PS([128, 128])
            nc.tensor.matmul(ps[:], ar0[:], br0[:], start=True, stop=False)
            nc.tensor.matmul(ps[:], ar1[:], br1[:], start=False, stop=True)
            out = T([128, 128], tag=tag)
            v.tensor_sub(out[:], dg[:], ps[:])
            return out

        blkA1 = blk_build("blkA1", cols[:, 0:1], cols[:, 1:2],
                          a1r0, a1r1, pri, nir)
        blkA0H = blk_build("blkA0H", cols[:, 2:3], cols[:, 3:4],
                           a0r0, a0r1, b0r0, b0r1)

        # ---- Abar pair, b2, squaring chain + V/C doubling ---------------
        A2 = [None] * (NSQ + 1)
        A2T = [None] * (NSQ + 1)
        ps1 = PS([128, 256])
        nc.tensor.matmul(ps1[:, 0:128], blkA0H[:], blkA1[:],
                         start=True, stop=True)
        nc.tensor.matmul(ps1[:, 128:256], blkA1[:], blkA0H[:],
                         start=True, stop=True)
        A2[0] = T([128, 128], tag="A2_0")
        v.tensor_copy(A2[0][:], ps1[:, 0:128])
        A2T[0] = T([128, 128], tag="A2T_0")
        s.copy(A2T[0][:], ps1[:, 128:256])
        Vbuf = C([128, 132], tag="Vbuf")
        ps2 = PS([128, 1])
        nc.tensor.matmul(ps2[:], blkA0H[:], cols[:, 4:5], start=True, stop=True)
        v.tensor_copy(Vbuf[:, 128:129], ps2[:])        # b2
        Ccol = C([128, 128], tag="Ccol")
        v.tensor_copy(Ccol[:, 0:1], cols[:, 5:6])
        psv0 = PS([128, 1])
        nc.tensor.matmul(psv0[:], A2T[0][:], Vbuf[:, 128:129],
                         start=True, stop=True)
        v.tensor_copy(Vbuf[:, 0:1], psv0[:])           # Abar b2

        for k in range(NSQ):
            pssq = PS([128, 256])
            nc.tensor.matmul(pssq[:, 0:128], A2T[k][:], A2[k][:],
                             start=True, stop=True)
            nc.tensor.matmul(pssq[:, 128:256], A2[k][:], A2T[k][:],
                             start=True, stop=True)
            if k < 7:
                w_ = 1 << k
                psv = PS([128, 64], tag="pp_psv")
                nc.tensor.matmul(psv[:, 0:w_], A2T[k][:], Vbuf[:, 0:w_],
                                 start=True, stop=True)
                psck = PS([128, 64], tag="pp_psc")
                nc.tensor.matmul(psck[:, 0:w_], A2[k][:], Ccol[:, 0:w_],
                                 start=True, stop=True)
            A2[k + 1] = T([128, 128], tag=f"A2_{k+1}")
            v.tensor_copy(A2[k + 1][:], pssq[:, 0:128])
            A2T[k + 1] = T([128, 128], tag=f"A2T_{k+1}")
            s.copy(A2T[k + 1][:], pssq[:, 128:256])
            if k < 7:
                w_ = 1 << k
                v.tensor_copy(Vbuf[:, w_:2 * w_], psv[:, 0:w_])
                s.copy(Ccol[:, w_:2 * w_], psck[:, 0:w_])

        # ---- scan maps --------------------------------------------------
        def bf_map(tag):
            return con.tile([128, 128], BF16, tag=tag, name=tag)

        # near-field Toeplitz T0: K row via DRAM shift, D on diagonal
        psk = PS([1, 128], tag="pp_psk")
        nc.tensor.matmul(psk[:], Vbuf[:, 128:129], Ccol[:],
                         start=True, stop=True)
        v.tensor_add(psk[0:1, 0:1], psk[0:1, 0:1], dval[:])   # += D
        zrow = T([1, 128], BF16, tag="zrow")
        v.tensor_copy(zrow[:], psk[:])
        nc.scalar.dma_start(out=zs_d[0:1, 128:256], in_=zrow[:])
        T0R = con.tile([128, 128], BF16, tag="T0R", name="T0R")
        zsap = zs_d[:]
        src = bass.AP(zsap.tensor, 1, [[1, 128], [1, 128]])
        nc.scalar.dma_start(out=T0R[:], in_=src)    # T0R[p,t] = zs[1+p+t]
        pst0 = PS([128, 128])
        nc.tensor.matmul(pst0[:], rev_bf[:], T0R[:], start=True, stop=True)
        T0_bf = bf_map("T0_bf")
        v.tensor_copy(T0_bf[:], pst0[:])            # T0[p,t] = K[t-p]

        pst = PS([128, 128])
        nc.tensor.transpose(pst[:], Vbuf[:, 0:128], ident[:])
        VT = T([128, 128], tag="VT")
        v.tensor_copy(VT[:], pst[:])
        psmm = PS([128, 256])
        nc.tensor.matmul(psmm[:, 0:128], rev[:], VT[:], start=True, stop=True)
        nc.tensor.matmul(psmm[:, 128:256], VT[:], rev[:], start=True, stop=True)
        MinT_bf = bf_map("MinT_bf")
        v.tensor_copy(MinT_bf[:], psmm[:, 0:128])
        Min = T([128, 128], tag="Min")
        s.copy(Min[:], psmm[:, 128:256])
        psmg = PS([128, 256])
        nc.tensor.matmul(psmg[:, 0:128], Min[:], A2T[7][:],
                         start=True, stop=True)
        nc.tensor.matmul(psmg[:, 128:256], Min[:], Ccol[:],
                         start=True, stop=True)
        ET_bf = bf_map("ET_bf")
        v.tensor_copy(ET_bf[:], psmg[:, 0:128])
        G0_bf = bf_map("G0_bf")
        s.copy(G0_bf[:], psmg[:, 128:256])
        psw1 = PS([128, 128])
        nc.tensor.matmul(psw1[:], A2[7][:], Ccol[:], start=True, stop=True)
        W1_bf = bf_map("W1_bf")
        v.tensor_copy(W1_bf[:], psw1[:])
        Wout_bf = bf_map("Wout_bf")
        s.copy(Wout_bf[:], Ccol[:])
        Dq2T_bf = bf_map("Dq2T_bf")
        s.copy(Dq2T_bf[:], A2T[NSQ][:])

        # ================= main loop =====================================
        pps_ctx.__exit__(None, None, None)   # release prefix PSUM banks
        hp = self.pool("h", 2)
        yp = self.pool("yt", 2)
        ph_p = self.pool("ph", 2, "PSUM")
        py_p = self.pool("py", 4, "PSUM")
        y_r = yT_d.rearrange("q (i b) -> q i b", b=BC)
        mm = nc.tensor.matmul

        h_in = None
        for gi in range(NG):
            c = [4 * gi + j for j in range(4)]
            py = [py_p.tile([128, BC], F32, tag="py", name="py")
                  for _ in range(4)]
            # near field, one stationary load
            for j in range(4):
                mm(py[j][:], T0_bf[:], usb[:, c[j], :], start=True,
                   stop=(j == 0 and h_in is None))
            mm(py[1][:], G0_bf[:], usb[:, c[0], :], start=False,
               stop=(h_in is None))
            mm(py[3][:], G0_bf[:], usb[:, c[2], :], start=False, stop=False)
            # state accumulation (u terms)
            last = gi == NG - 1
            ph1 = ph_p.tile([128, BC], F32, tag="ph", name="ph")
            mm(ph1[:], ET_bf[:], usb[:, c[0], :], start=True, stop=False)
            if not last:
                ph2 = ph_p.tile([128, BC], F32, tag="ph", name="ph")
                mm(ph2[:], ET_bf[:], usb[:, c[2], :], start=True, stop=False)
            mm(ph1[:], MinT_bf[:], usb[:, c[1], :], start=False,
               stop=(h_in is None))
            if not last:
                mm(ph2[:], MinT_bf[:], usb[:, c[3], :], start=False,
                   stop=False)
            if h_in is not None:
                mm(ph1[:], Dq2T_bf[:], h_in[:], start=False, stop=True)
                mm(py[0][:], Wout_bf[:], h_in[:], start=False, stop=True)
                mm(py[1][:], W1_bf[:], h_in[:], start=False, stop=True)
            # mid-state h_{2g+1}: split eviction for latency
            hb = hp.tile([128, BC], BF16, tag="h", name="h")
            v.tensor_copy(hb[:, 0:256], ph1[:, 0:256])
            s.copy(hb[:, 256:512], ph1[:, 256:512])
            if not last:
                mm(ph2[:], Dq2T_bf[:], hb[:], start=False, stop=True)
            mm(py[2][:], Wout_bf[:], hb[:], start=False, stop=True)
            mm(py[3][:], W1_bf[:], hb[:], start=False, stop=True)
            if not last:
                h_out = hp.tile([128, BC], BF16, tag="h", name="h")
                v.tensor_copy(h_out[:, 0:256], ph2[:, 0:256])
                s.copy(h_out[:, 256:512], ph2[:, 256:512])
            # evictions spread across engines; one store per group
            yt = yp.tile([128, 4, BC], BF16, tag="yt", name="yt")
            v.tensor_copy(yt[:, 0, :], py[0][:])
            s.copy(yt[:, 1, :], py[1][:])
            v.tensor_copy(yt[:, 2, :], py[2][:])
            s.copy(yt[:, 3, :], py[3][:])
            nc.scalar.dma_start(out=y_r[:, 4 * gi:4 * gi + 4, :], in_=yt[:])
            if not last:
                h_in = h_out


def _consts():
    ident = np.eye(128, dtype=np.float32)
    ilmu = np.zeros((128, 128), dtype=np.float32)
    for p in range(64):
        ilmu[p, p + 64] = -1.0
        ilmu[p + 64, p] = 1.0
    rev = ident[::-1].copy()
    return np.concatenate([ident, ilmu, rev], axis=1)


def kernel(**inputs):
    global LAST_EXEC_NS, LAST_RESULTS
    nc = build_program()
    f32 = np.float32
    pk = np.zeros((1, PKLEN), f32)
    pre = inputs["P_re"].astype(f32).ravel()
    pim = inputs["P_im"].astype(f32).ravel()
    pk[0, PK_LRI:PK_LRI + 128] = np.concatenate(
        [inputs["Lambda_re"].astype(f32).ravel(),
         inputs["Lambda_im"].astype(f32).ravel()])
    pk[0, PK_PRI:PK_PRI + 128] = np.concatenate([pre, pim])
    pk[0, PK_PIR:PK_PIR + 128] = np.concatenate([pim, pre])
    pk[0, PK_NIR:PK_NIR + 128] = np.concatenate([-pim, pre])
    pk[0, PK_BRI:PK_BRI + 128] = np.concatenate(
        [inputs["B_re"].astype(f32).ravel(),
         inputs["B_im"].astype(f32).ravel()])
    cri = inputs["C_ri"].astype(f32)
    pk[0, PK_CRI:PK_CRI + 128] = np.concatenate([cri[:, 0], cri[:, 1]])
    pk[0, PK_E1:PK_E1 + 64] = 1.0
    pk[0, PK_D] = f32(inputs["D"].ravel()[0])
    pk[0, PK_LSTEP] = f32(inputs["log_step"].ravel()[0])
    pk[0, PK_ONE] = 1.0

    base = {
        "pk": pk,
        "cst": _consts(),
        "zs": np.zeros((1, 256), ml_dtypes.bfloat16),
    }
    u = np.asarray(inputs["u"], dtype=f32)
    in_maps = []
    for cix in range(NCORES):
        m = dict(base)
        uc = u[cix * BC:(cix + 1) * BC]          # [512, 4096]
        m["uT"] = np.ascontiguousarray(
            uc.T.reshape(NCH, 128, BC).transpose(1, 0, 2).reshape(
                128, NCH * BC)).astype(ml_dtypes.bfloat16)
        in_maps.append(m)
    trace = bool(int(os.environ.get("KERNEL_TRACE", "0")))
    kw = {}
    if trace:
        kw["trace"] = True
        kw["trace_cores"] = list(range(NCORES))
    res = run_bass_kernel_spmd(nc, in_maps, list(range(NCORES)), **kw)
    LAST_EXEC_NS = res.exec_time_ns
    LAST_RESULTS = res
    outs = []
    for r in res.results:
        a = np.asarray(r["yT"]).reshape(128, NCH, BC).transpose(1, 0, 2)
        outs.append(a.reshape(L, BC).astype(f32).T)
    return np.concatenate(outs, axis=0)


# revision 21
# speedup vs baseline: 1.1469x; 1.1469x over previous
"""Trainium2 Bass kernel for nn_LiquidS4Layer (S4 DPLR forward).

y = causal_conv(u, K) + D*u, K the length-L SSM kernel from small DPLR params.

Device algorithm (per core, 512 of 4096 batch rows):
  1. Discretize via bilinear transform + Woodbury (A = Lambda - P P^H is
     diagonal + rank-1) -> block-real forms blkA1, blkA0H; Abar pair by mm.
  2. Repeated squaring Abar^(2^k), k<=8; V-chain (Abar^j Bbar) and C-chain
     ((Abar^T)^j Ctilde) by doubling -> scan maps Min/E/G0/W1/Wout/Dq2 and
     the near-field Toeplitz T0 (K row via DRAM shift trick, D on diagonal).
     Aliasing correction is skipped: |eig(Abar)|^L <= 2.3e-2 and the measured
     end-to-end impact is < 2e-4 relative.
  3. Chunked scan (chunk Q=128, stride-2 state passing), computing y^T:
     all matmuls put the small [128,128] map as the stationary operand and
     u/h [128,512] as the moving operand; matmuls are grouped by stationary
     operand (4 chunks per group) and repeated weight loads are elided.
Host side does layout only: packs small params, pre-transposes + bf16-casts
the u shard into a [q, chunk, b] DRAM layout (contiguous per partition),
and relayouts the bf16 y^T shard back to f32 [b, t].

Sharding: u/(y) row-sharded over 8 cores (batch*channel parallel); params
replicated; no collectives.
"""
import os
import numpy as np
import ml_dtypes
from contextlib import ExitStack

import concourse.bass as bass
import concourse.tile as tile
from concourse import mybir
from concourse.bass_utils import run_bass_kernel_spmd

F32 = mybir.dt.float32
BF16 = mybir.dt.bfloat16
ALU = mybir.AluOpType

NCORES = 8
BH, L = 4096, 4096
BC = BH // NCORES       # 512 rows per core
N = 64                  # SSM state size
N2 = 2 * N              # real block state size = 128
Q = 128                 # chunk length
NCH = L // Q            # 32 chunks
NSQ = 8                 # Abar^(2^8) = Abar^256 = Dq^2
NG = NCH // 4           # 8 four-chunk groups

# packed-param layout (f32 row)
PK_LRI, PK_PRI, PK_PIR, PK_NIR = 0, 128, 256, 384
PK_E1, PK_ONES = 512, 640
PK_D, PK_LSTEP, PK_ONE = 768, 769, 770
PKLEN = 772
# consts layout [128, CSTW]: ident | E1d | E2d | rev | 6 columns
CST_ID, CST_E1D, CST_E2D, CST_REV = 0, 128, 256, 384
CST_LC, CST_LCS, CST_E1C, CST_E1CS, CST_BC, CST_CC = 512, 513, 514, 515, 516, 517
CSTW = 518

LAST_EXEC_NS = None
LAST_RESULTS = None


def build_program():
    nc = bass.Bass()
    dp = nc.declare_dram_parameter
    uT = dp("uT", [128, NCH * BC], BF16, isOutput=False)
    yT = dp("yT", [128, NCH * BC], BF16, isOutput=True)
    pk = dp("pk", [1, PKLEN], F32, isOutput=False)
    cst = dp("cst", [128, CSTW], F32, isOutput=False)
    zs = dp("zs", [1, 256], BF16, isOutput=False)

    with TileKernel(nc) as tk:
        tk.build(uT, yT, pk, cst, zs)
    if not int(os.environ.get("KERNEL_NO_LDW_DEDUPE", "0")):
        _dedupe_ldweights(nc)
    _split_multi_waits(nc)
    return nc


def _dedupe_ldweights(nc):
    """Mark matmuls whose stationary operand matches the immediately
    preceding matmul's so codegen skips the redundant LDWEIGHTS.  fp32
    matmuls are excluded (two-pass weight load is fused with the mm)."""
    for f in nc.m.functions:
        for blk in f.blocks:
            last = None
            for inst in blk.instructions:
                if not isinstance(inst, mybir.InstMatmult):
                    continue
                w = inst.ins[1]
                if inst.is_transpose or w.dtype in (mybir.dt.float32,
                                                    mybir.dt.float32r):
                    last = None
                    continue
                key = (w.memref, w.offset, str(w.ap))
                if last is not None and key == last:
                    inst.ldweights = False
                last = key


def _split_multi_waits(nc):
    """This toolchain's walrus encodes at most one sync wait per (non-Drain)
    instruction.  Tile can emit several; hoist the extras onto standalone
    EventSemaphore wait instructions inserted just before, on the same
    engine (engines execute their stream in order, so this is equivalent)."""
    ctr = 0
    for f in nc.m.functions:
        for blk in f.blocks:
            out = []
            changed = False
            for inst in blk.instructions:
                si = inst.sync_info
                if si is None:
                    out.append(inst)
                    continue
                waits = list(si.on_wait)
                if len(waits) > 1:
                    cands = [u for u in si.on_update] + [
                        w for w in waits if "DMA" not in w.ant_name]
                    for w in waits[:-1]:
                        ev = mybir.InstEventSemaphore(
                            name=f"I-wsplit-{ctr}", ins=[], outs=[])
                        ctr += 1
                        c = cands[0] if cands else w
                        ev.engine = inst.engine
                        up = mybir.SyncUpdate(
                            sync_type="semaphore", id=c.id, ant_name=c.ant_name,
                            update_mode="sem-add-imm", update_value=0,
                            update_reg=None)
                        ev.sync_info = mybir.SyncInfo(on_wait=[w], on_update=[up])
                        out.append(ev)
                    inst.sync_info = mybir.SyncInfo(
                        on_wait=[waits[-1]], on_update=list(si.on_update))
                    changed = True
                out.append(inst)
            if changed:
                blk.instructions = out


class TileKernel:
    def __init__(self, nc):
        self.nc = nc
        self.ctx = ExitStack()
        self.tc = tile.TileContext(nc)

    def __enter__(self):
        self.ctx.__enter__()
        self.tc.__enter__()
        return self

    def __exit__(self, *a):
        self.ctx.__exit__(*a)
        return self.tc.__exit__(*a)

    def pool(self, name, bufs=1, space="SBUF"):
        return self.ctx.enter_context(
            self.tc.tile_pool(name=name, bufs=bufs, space=space))

    def build(self, uT_d, yT_d, pk_d, cst_d, zs_d):
        nc = self.nc
        v = nc.vector
        s = nc.scalar
        con = self.pool("con", 1)
        pp = self.pool("pp", 1)
        pps_ctx = self.tc.tile_pool(name="pps", bufs=2, space="PSUM")
        pps = pps_ctx.__enter__()

        def T(shape, dt=F32, tag=None):
            return pp.tile(shape, dt, tag=tag, name=tag)

        def C(shape, dt=F32, tag=None):
            return con.tile(shape, dt, tag=tag, name=tag)

        def PS(shape, tag="pp_ps"):
            return pps.tile(shape, F32, tag=tag, name=tag)

        # ---- t=0: ACT table preload + PE warm-up burst ------------------
        dzb = C([128, 640], BF16, tag="dzb")
        v.memset(dzb[:], 0.0)
        dact = T([1, 1], tag="dact")
        s.activation(dact[:], dzb[0:1, 0:1],
                     mybir.ActivationFunctionType.Exp)
        for _ in range(9):
            psd = PS([128, 512], tag="pp_ps")
            nc.tensor.matmul(psd[:], dzb[:, 512:640], dzb[:, 0:512],
                             start=True, stop=True)

        # ---- loads ------------------------------------------------------
        pk = C([1, PKLEN], tag="pk")
        nc.sync.dma_start(out=pk[:], in_=pk_d[:])
        cst = C([128, CSTW], tag="cst")
        nc.sync.dma_start(out=cst[:], in_=cst_d[:])
        ident = cst[:, CST_ID:CST_ID + 128]
        e1d = cst[:, CST_E1D:CST_E1D + 128]
        e2d = cst[:, CST_E2D:CST_E2D + 128]
        rev = cst[:, CST_REV:CST_REV + 128]
        lcol = cst[:, CST_LC:CST_LC + 1]
        lcol_sw = cst[:, CST_LCS:CST_LCS + 1]
        e1col = cst[:, CST_E1C:CST_E1C + 1]
        e1col_sw = cst[:, CST_E1CS:CST_E1CS + 1]
        bcol = cst[:, CST_BC:CST_BC + 1]
        ccol = cst[:, CST_CC:CST_CC + 1]
        usb = C([128, NCH, BC], BF16, tag="usb")
        uT_r = uT_d.rearrange("q (i b) -> q i b", b=BC)
        for p in range(4):
            nc.sync.dma_start(out=usb[:, p * 8:(p + 1) * 8, :],
                              in_=uT_r[:, p * 8:(p + 1) * 8, :])
        rev_bf = C([128, 128], BF16, tag="rev_bf")
        s.copy(rev_bf[:], rev[:])

        lri = pk[0:1, PK_LRI:PK_LRI + 128]
        pri = pk[0:1, PK_PRI:PK_PRI + 128]
        pir = pk[0:1, PK_PIR:PK_PIR + 128]
        nir = pk[0:1, PK_NIR:PK_NIR + 128]
        e1 = pk[0:1, PK_E1:PK_E1 + 128]          # [1]*64 + [0]*64
        ones = pk[0:1, PK_ONES:PK_ONES + 128]
        dval = pk[0:1, PK_D:PK_D + 1]
        lstep = pk[0:1, PK_LSTEP:PK_LSTEP + 1]
        one11 = pk[0:1, PK_ONE:PK_ONE + 1]

        # ---- param chain: vector spine + scalar leaves ------------------
        delta = T([1, 1], tag="delta")
        s.activation(delta[:], lstep[:], mybir.ActivationFunctionType.Exp)
        hh = T([1, 1], tag="hh")
        v.tensor_scalar_mul(hh[:], delta[:], 0.5)
        psh = PS([128, 1], tag="pp_ps")
        nc.tensor.matmul(psh[:], ones[:], hh[:], start=True, stop=True)
        hhcol = T([128, 1], tag="hhcol")
        v.tensor_copy(hhcol[:], psh[:])
        nhhcol = T([128, 1], tag="nhhcol")
        v.tensor_scalar_mul(nhhcol[:], hhcol[:], -1.0)
        dcol = T([128, 1], tag="dcol")
        v.tensor_scalar_mul(dcol[:], hhcol[:], 2.0)
        hl = T([1, 128], tag="hl")
        v.tensor_scalar_mul(hl[:], lri[:], hh[:])
        den = T([1, 128], tag="den")             # 1 - hh*lam
        v.scalar_tensor_tensor(den[:], hl[:], -1.0, e1[:], ALU.mult, ALU.add)
        sq = T([1, 128], tag="sq")
        v.tensor_mul(sq[:], den[:], den[:])
        r2 = T([1, 64], tag="r2")
        v.tensor_add(r2[:], sq[0:1, 0:64], sq[0:1, 64:128])
        rr = T([1, 128], tag="rr")
        v.reciprocal(rr[0:1, 0:64], r2[:])
        v.tensor_copy(rr[0:1, 64:128], rr[0:1, 0:64])
        d0 = T([1, 128], tag="d0")               # 1/den (conj trick)
        v.tensor_mul(d0[0:1, 0:64], den[0:1, 0:64], rr[0:1, 0:64])
        v.scalar_tensor_tensor(d0[0:1, 64:128], den[0:1, 64:128], -1.0,
                               rr[0:1, 64:128], ALU.mult, ALU.mult)
        psq = T([1, 128], tag="psq")
        v.tensor_mul(psq[:], pri[:], pri[:])
        p2 = T([1, 64], tag="p2")
        v.tensor_add(p2[:], psq[0:1, 0:64], psq[0:1, 64:128])
        m1 = T([1, 128], tag="m1")
        v.tensor_mul(m1[:], d0[:], pri[:])
        m2 = T([1, 128], tag="m2")
        v.tensor_mul(m2[:], d0[:], pir[:])
        tri = T([1, 128], tag="tri")             # [tre | tim], t = d0*P
        v.tensor_sub(tri[0:1, 0:64], m1[0:1, 0:64], m1[0:1, 64:128])
        v.tensor_add(tri[0:1, 64:128], m2[0:1, 0:64], m2[0:1, 64:128])
        # a0 rows: a = conj(v), v = conj(P)*d0
        a0r0 = T([1, 128], tag="a0r0")           # [vre | -vim]
        a0r1 = T([1, 128], tag="a0r1")           # [vim | vre]
        v.tensor_add(a0r0[0:1, 0:64], m1[0:1, 0:64], m1[0:1, 64:128])
        v.tensor_sub(a0r1[0:1, 0:64], m2[0:1, 64:128], m2[0:1, 0:64])
        s.mul(a0r0[0:1, 64:128], a0r1[0:1, 0:64], -1.0)
        s.copy(a0r1[0:1, 64:128], a0r0[0:1, 0:64])
        # s_c = 1 + hh * sum(|P|^2 d0)
        pd = T([1, 128], tag="pd")
        sri = T([1, 2], tag="sri")
        v.tensor_mul(pd[0:1, 0:64], d0[0:1, 0:64], p2[:])
        v.tensor_mul(pd[0:1, 64:128], d0[0:1, 64:128], p2[:])
        v.reduce_sum(sri[0:1, 0:1], pd[0:1, 0:64], axis=mybir.AxisListType.X)
        v.reduce_sum(sri[0:1, 1:2], pd[0:1, 64:128], axis=mybir.AxisListType.X)
        sc = T([1, 2], tag="sc")                 # [s_re | s_im]
        v.tensor_scalar(sc[0:1, 0:1], sri[0:1, 0:1], hh[:], 1.0,
                        op0=ALU.mult, op1=ALU.add)
        v.tensor_scalar_mul(sc[0:1, 1:2], sri[0:1, 1:2], hh[:])
        sq2 = T([1, 2], tag="sq2")
        v.tensor_mul(sq2[:], sc[:], sc[:])
        s2 = T([1, 1], tag="s2")
        v.tensor_add(s2[:], sq2[0:1, 0:1], sq2[0:1, 1:2])
        s2i = T([1, 1], tag="s2i")
        v.reciprocal(s2i[:], s2[:])
        hsr = T([1, 1], tag="hsr")               # hh*s_re*s2i
        v.scalar_tensor_tensor(hsr[:], sc[0:1, 0:1], hh[:], s2i[:],
                               ALU.mult, ALU.mult)
        hsi = T([1, 1], tag="hsi")               # +hh*s_im*s2i (sign folded)
        v.scalar_tensor_tensor(hsi[:], sc[0:1, 1:2], hh[:], s2i[:],
                               ALU.mult, ALU.mult)
        # w = hs * t -> b0 rows (b~ = w); hs = hsr - i*hsi
        x2a = T([1, 64], tag="x2a")
        v.tensor_scalar_mul(x2a[:], tri[0:1, 64:128], hsi[:])
        x2b = T([1, 64], tag="x2b")
        v.tensor_scalar_mul(x2b[:], tri[0:1, 0:64], hsi[:])
        b0r0 = T([1, 128], tag="b0r0")           # [wre | wim]
        b0r1 = T([1, 128], tag="b0r1")           # [-wim | wre]
        v.scalar_tensor_tensor(b0r0[0:1, 0:64], tri[0:1, 0:64], hsr[:],
                               x2a[:], ALU.mult, ALU.add)
        v.scalar_tensor_tensor(b0r0[0:1, 64:128], tri[0:1, 64:128], hsr[:],
                               x2b[:], ALU.mult, ALU.subtract)
        s.mul(b0r1[0:1, 0:64], b0r0[0:1, 64:128], -1.0)
        s.copy(b0r1[0:1, 64:128], b0r0[0:1, 0:64])
        # a1 rows: a = hh*P ; b1 rows are pri / nir (host-packed)
        a1r0 = T([1, 128], tag="a1r0")
        v.tensor_scalar_mul(a1r0[:], pri[:], hh[:])
        a1r1 = T([1, 128], tag="a1r1")
        s.mul(a1r1[0:1, 0:64], a1r0[0:1, 64:128], -1.0)
        s.copy(a1r1[0:1, 64:128], a1r0[0:1, 0:64])
        # diag columns from host-packed lcol/lcol_sw (+ rinv via PE)
        c1 = T([128, 1], tag="c1")               # [1+hh*lre ; hh*lim]
        v.scalar_tensor_tensor(c1[:], lcol[:], hhcol[:], e1col[:],
                               ALU.mult, ALU.add)
        c1s = T([128, 1], tag="c1s")             # [hh*lim ; 1+hh*lre]
        v.scalar_tensor_tensor(c1s[:], lcol_sw[:], hhcol[:], e1col_sw[:],
                               ALU.mult, ALU.add)
        dnc = T([128, 1], tag="dnc")             # [1-hh*lre ; -hh*lim]
        v.scalar_tensor_tensor(dnc[:], lcol[:], nhhcol[:], e1col[:],
                               ALU.mult, ALU.add)
        dncs = T([128, 1], tag="dncs")           # [-hh*lim ; 1-hh*lre]
        v.scalar_tensor_tensor(dncs[:], lcol_sw[:], nhhcol[:], e1col_sw[:],
                               ALU.mult, ALU.add)
        psr = PS([128, 1], tag="pp_ps")
        nc.tensor.matmul(psr[:], rr[:], one11[:], start=True, stop=True)
        rcol = T([128, 1], tag="rcol")           # [rinv ; rinv]
        v.tensor_copy(rcol[:], psr[:])
        c0 = T([128, 1], tag="c0")               # conj(d0) col
        v.tensor_mul(c0[:], dnc[:], rcol[:])
        c0s = T([128, 1], tag="c0s")
        v.tensor_mul(c0s[:], dncs[:], rcol[:])

        # ---- block matrices --------------------------------------------
        def blk_build(tag, c, c_sw, ar0, ar1, br0, br1):
            dg = T([128, 128], tag=tag + "_dg")
            v.tensor_scalar_mul(dg[:], e1d[:], c)
            dgi = T([128, 128], tag=tag + "_dgi")
            v.tensor_scalar_mul(dgi[:], e2d[:], c_sw)
            v.tensor_add(dg[:], dg[:], dgi[:])
            ps = PS([128, 128])
            nc.tensor.matmul(ps[:], ar0[:], br0[:], start=True, stop=False)
            nc.tensor.matmul(ps[:], ar1[:], br1[:], start=False, stop=True)
            out = T([128, 128], tag=tag)
            v.tensor_sub(out[:], dg[:], ps[:])
            return out

        blkA1 = blk_build("blkA1", c1[:], c1s[:], a1r0, a1r1, pri, nir)
        blkA0H = blk_build("blkA0H", c0[:], c0s[:], a0r0, a0r1, b0r0, b0r1)

        # ---- Abar pair, b2, squaring chain + V/C doubling ---------------
        A2 = [None] * (NSQ + 1)
        A2T = [None] * (NSQ + 1)
        ps1 = PS([128, 256])
        nc.tensor.matmul(ps1[:, 0:128], blkA0H[:], blkA1[:],
                         start=True, stop=True)
        nc.tensor.matmul(ps1[:, 128:256], blkA1[:], blkA0H[:],
                         start=True, stop=True)
        A2[0] = T([128, 128], tag="A2_0")
        v.tensor_copy(A2[0][:], ps1[:, 0:128])
        A2T[0] = T([128, 128], tag="A2T_0")
        s.copy(A2T[0][:], ps1[:, 128:256])
        Vbuf = C([128, 132], tag="Vbuf")
        ps2 = PS([128, 1])
        nc.tensor.matmul(ps2[:], blkA0H[:], bcol[:], start=True, stop=True)
        v.tensor_copy(Vbuf[:, 128:129], ps2[:])        # b2 (delta folded later)
        Ccol = C([128, 128], tag="Ccol")
        v.tensor_copy(Ccol[:, 0:1], ccol[:])
        psv0 = PS([128, 1])
        nc.tensor.matmul(psv0[:], A2T[0][:], Vbuf[:, 128:129],
                         start=True, stop=True)
        v.tensor_copy(Vbuf[:, 0:1], psv0[:])           # Abar b2

        for k in range(NSQ - 1):                       # waves 0..6
            pssq = PS([128, 256])
            nc.tensor.matmul(pssq[:, 0:128], A2T[k][:], A2[k][:],
                             start=True, stop=True)
            nc.tensor.matmul(pssq[:, 128:256], A2[k][:], A2T[k][:],
                             start=True, stop=True)
            w_ = 1 << k
            psv = PS([128, 64], tag="pp_psv")
            nc.tensor.matmul(psv[:, 0:w_], A2T[k][:], Vbuf[:, 0:w_],
                             start=True, stop=True)
            psck = PS([128, 64], tag="pp_psc")
            nc.tensor.matmul(psck[:, 0:w_], A2[k][:], Ccol[:, 0:w_],
                             start=True, stop=True)
            A2[k + 1] = T([128, 128], tag=f"A2_{k+1}")
            v.tensor_copy(A2[k + 1][:], pssq[:, 0:128])
            A2T[k + 1] = T([128, 128], tag=f"A2T_{k+1}")
            s.copy(A2T[k + 1][:], pssq[:, 128:256])
            v.tensor_copy(Vbuf[:, w_:2 * w_], psv[:, 0:w_])
            s.copy(Ccol[:, w_:2 * w_], psck[:, 0:w_])

        # ---- scan maps (before the last squaring wave) ------------------
        def bf_map(tag):
            return con.tile([128, 128], BF16, tag=tag, name=tag)

        # near-field Toeplitz T0: K row via DRAM shift, delta & D folded
        psk = PS([1, 128], tag="pp_psk")
        nc.tensor.matmul(psk[:], Vbuf[:, 128:129], Ccol[:],
                         start=True, stop=True)
        v.tensor_scalar_mul(psk[:], psk[:], delta[:])
        v.tensor_add(psk[0:1, 0:1], psk[0:1, 0:1], dval[:])     # += D
        zrow = T([1, 128], BF16, tag="zrow")
        v.tensor_copy(zrow[:], psk[:])
        nc.scalar.dma_start(out=zs_d[0:1, 128:256], in_=zrow[:])
        T0R = con.tile([128, 128], BF16, tag="T0R", name="T0R")
        zsap = zs_d[:]
        src = bass.AP(zsap.tensor, 1, [[1, 128], [1, 128]])
        nc.scalar.dma_start(out=T0R[:], in_=src)    # T0R[p,t] = zs[1+p+t]
        pst = PS([128, 128])
        nc.tensor.transpose(pst[:], Vbuf[:, 0:128], ident[:])
        VT = T([128, 128], tag="VT")
        v.tensor_copy(VT[:], pst[:])
        psmm = PS([128, 256])
        nc.tensor.matmul(psmm[:, 0:128], rev[:], VT[:], start=True, stop=True)
        nc.tensor.matmul(psmm[:, 128:256], VT[:], rev[:], start=True, stop=True)
        MinT_bf = bf_map("MinT_bf")
        v.tensor_scalar_mul(MinT_bf[:], psmm[:, 0:128], dcol[:])
        Min = T([128, 128], tag="Min")
        v.tensor_scalar_mul(Min[:], psmm[:, 128:256], dcol[:])
        psmg = PS([128, 256])
        nc.tensor.matmul(psmg[:, 0:128], Min[:], A2T[7][:],
                         start=True, stop=True)
        nc.tensor.matmul(psmg[:, 128:256], Min[:], Ccol[:],
                         start=True, stop=True)
        ET_bf = bf_map("ET_bf")
        v.tensor_copy(ET_bf[:], psmg[:, 0:128])
        G0_bf = bf_map("G0_bf")
        s.copy(G0_bf[:], psmg[:, 128:256])
        psw1 = PS([128, 128])
        nc.tensor.matmul(psw1[:], A2[7][:], Ccol[:], start=True, stop=True)
        W1_bf = bf_map("W1_bf")
        v.tensor_copy(W1_bf[:], psw1[:])
        Wout_bf = bf_map("Wout_bf")
        s.copy(Wout_bf[:], Ccol[:])
        pst0 = PS([128, 128])
        nc.tensor.matmul(pst0[:], rev_bf[:], T0R[:], start=True, stop=True)
        T0_bf = bf_map("T0_bf")
        v.tensor_copy(T0_bf[:], pst0[:])            # T0[p,t] = K[t-p]
        # last squaring wave: Dq^2 = Abar^256 (transposed only)
        pssq = PS([128, 128])
        nc.tensor.matmul(pssq[:], A2[NSQ - 1][:], A2T[NSQ - 1][:],
                         start=True, stop=True)
        Dq2T_bf = bf_map("Dq2T_bf")
        v.tensor_copy(Dq2T_bf[:], pssq[:])

        # ================= main loop =====================================
        pps_ctx.__exit__(None, None, None)   # release prefix PSUM banks
        hp = self.pool("h", 2)
        yp = self.pool("yt", 3)
        ph_p = self.pool("ph", 2, "PSUM")
        py_p = self.pool("py", 4, "PSUM")
        y_r = yT_d.rearrange("q (i b) -> q i b", b=BC)
        mm = nc.tensor.matmul

        h_in = None
        for gi in range(NG):
            c = [4 * gi + j for j in range(4)]
            first = h_in is None
            last = gi == NG - 1
            py = [py_p.tile([128, BC], F32, tag="py", name="py")
                  for _ in range(4)]
            for j in range(4):
                mm(py[j][:], T0_bf[:], usb[:, c[j], :], start=True,
                   stop=(j == 0 and first))
            ph1 = ph_p.tile([128, BC], F32, tag="ph", name="ph")
            mm(ph1[:], ET_bf[:], usb[:, c[0], :], start=True, stop=False)
            if not last:
                ph2 = ph_p.tile([128, BC], F32, tag="ph", name="ph")
                mm(ph2[:], ET_bf[:], usb[:, c[2], :], start=True, stop=False)
            mm(ph1[:], MinT_bf[:], usb[:, c[1], :], start=False, stop=first)
            if not last:
                mm(ph2[:], MinT_bf[:], usb[:, c[3], :], start=False,
                   stop=False)
            if not first:
                mm(ph1[:], Dq2T_bf[:], h_in[:], start=False, stop=True)
                mm(py[0][:], Wout_bf[:], h_in[:], start=False, stop=True)
                mm(py[1][:], W1_bf[:], h_in[:], start=False, stop=False)
            # mid-state h_{2g+1}: split eviction for latency
            hb = hp.tile([128, BC], BF16, tag="h", name="h")
            v.tensor_copy(hb[:, 0:256], ph1[:, 0:256])
            s.copy(hb[:, 256:512], ph1[:, 256:512])
            # u-only G0 terms cover the hb eviction latency
            mm(py[1][:], G0_bf[:], usb[:, c[0], :], start=False, stop=True)
            mm(py[3][:], G0_bf[:], usb[:, c[2], :], start=False, stop=False)
            if not last:
                mm(ph2[:], Dq2T_bf[:], hb[:], start=False, stop=True)
            mm(py[2][:], Wout_bf[:], hb[:], start=False, stop=True)
            mm(py[3][:], W1_bf[:], hb[:], start=False, stop=True)
            if not last:
                h_out = hp.tile([128, BC], BF16, tag="h", name="h")
                v.tensor_copy(h_out[:, 0:256], ph2[:, 0:256])
                s.copy(h_out[:, 256:512], ph2[:, 256:512])
            # evictions: promptness ordered; one store per group
            yt = yp.tile([128, 4, BC], BF16, tag="yt", name="yt")
            v.tensor_copy(yt[:, 0, :], py[0][:])
            s.copy(yt[:, 1, :], py[1][:])
            v.tensor_copy(yt[:, 2, :], py[2][:])
            s.copy(yt[:, 3, :], py[3][:])
            if last:
                nc.scalar.dma_start(out=y_r[:, 4 * gi:4 * gi + 2, :],
                                    in_=yt[:, 0:2, :])
                nc.scalar.dma_start(out=y_r[:, 4 * gi + 2:4 * gi + 4, :],
                                    in_=yt[:, 2:4, :])
            else:
                nc.scalar.dma_start(out=y_r[:, 4 * gi:4 * gi + 4, :],
                                    in_=yt[:])
            if not last:
                h_in = h_out


def _consts():
    ident = np.eye(128, dtype=np.float32)
    rev = ident[::-1].copy()
    e1d = np.zeros((128, 128), np.float32)
    e2d = np.zeros((128, 128), np.float32)
    r = np.arange(64)
    e1d[r, r] = 1.0
    e1d[r + 64, r] = 1.0
    e2d[r, r + 64] = -1.0
    e2d[r + 64, r + 64] = 1.0
    return ident, e1d, e2d, rev


def kernel(**inputs):
    global LAST_EXEC_NS, LAST_RESULTS
    nc = build_program()
    f32 = np.float32
    pk = np.zeros((1, PKLEN), f32)
    pre = inputs["P_re"].astype(f32).ravel()
    pim = inputs["P_im"].astype(f32).ravel()
    lre = inputs["Lambda_re"].astype(f32).ravel()
    lim = inputs["Lambda_im"].astype(f32).ravel()
    pk[0, PK_LRI:PK_LRI + 128] = np.concatenate([lre, lim])
    pk[0, PK_PRI:PK_PRI + 128] = np.concatenate([pre, pim])
    pk[0, PK_PIR:PK_PIR + 128] = np.concatenate([pim, pre])
    pk[0, PK_NIR:PK_NIR + 128] = np.concatenate([-pim, pre])
    pk[0, PK_E1:PK_E1 + 64] = 1.0
    pk[0, PK_ONES:PK_ONES + 128] = 1.0
    pk[0, PK_D] = f32(inputs["D"].ravel()[0])
    pk[0, PK_LSTEP] = f32(inputs["log_step"].ravel()[0])
    pk[0, PK_ONE] = 1.0

    ident, e1d, e2d, rev = _consts()
    cri = inputs["C_ri"].astype(f32)
    cols = np.zeros((128, 6), f32)
    cols[:, 0] = np.concatenate([lre, lim])                  # lcol
    cols[:, 1] = np.concatenate([lim, lre])                  # lcol_sw
    cols[0:64, 2] = 1.0                                      # e1col
    cols[64:128, 3] = 1.0                                    # e1col_sw
    cols[:, 4] = np.concatenate(
        [inputs["B_re"].astype(f32).ravel(),
         inputs["B_im"].astype(f32).ravel()])                # bcol (raw)
    cols[:, 5] = np.concatenate([cri[:, 0], cri[:, 1]])      # ccol
    cst = np.concatenate([ident, e1d, e2d, rev, cols], axis=1)

    base = {
        "pk": pk,
        "cst": cst,
        "zs": np.zeros((1, 256), ml_dtypes.bfloat16),
    }
    u = np.asarray(inputs["u"], dtype=f32)
    in_maps = []
    for cix in range(NCORES):
        m = dict(base)
        uc = u[cix * BC:(cix + 1) * BC]          # [512, 4096]
        m["uT"] = np.ascontiguousarray(
            uc.T.reshape(NCH, 128, BC).transpose(1, 0, 2).reshape(
                128, NCH * BC)).astype(ml_dtypes.bfloat16)
        in_maps.append(m)
    trace = bool(int(os.environ.get("KERNEL_TRACE", "0")))
    kw = {}
    if trace:
        kw["trace"] = True
        kw["trace_cores"] = list(range(NCORES))
    res = run_bass_kernel_spmd(nc, in_maps, list(range(NCORES)), **kw)
    LAST_EXEC_NS = res.exec_time_ns
    LAST_RESULTS = res
    outs = []
    for r in res.results:
        a = np.asarray(r["yT"]).reshape(128, NCH, BC).transpose(1, 0, 2)
        outs.append(a.reshape(L, BC).astype(f32).T)
    return np.concatenate(outs, axis=0)
